# revision 1
# baseline (speedup 1.0000x reference)
"""Trainium2 Bass kernel for nn_NumDualDescriptorAB.

Reference computation:
    agg[b,w]   = mean(seq[b, w:w+8, :], axis=0)          (sliding window, Nw = S-7)
    y[b,w]     = agg[b,w] @ M.T
    Nk[w]      = Acoeff[:, w%L] * Bbasis[w%L, :]
    D          = mean((y - Nk)^2)

Algebraic decomposition (everything heavy becomes matmuls with tiny outputs):
    count = B*Nw*m
    t1 = sum_{b,w} agg MtM agg^T = <M^T M, G>_F   with G = sum agg^T agg   (m x m)
    t2 = sum_{b,w} y . Nk = sum_{b,s} seq[b,s] . P[s]    with P = W^T (Nk M)  (S x m)
    t3 = B * ||Nk||^2
    D  = (t1 - 2 t2 + t3) / count

Device (8 cores, data-parallel over batch; 4 batches/core) computes G and
X^T = sum_chunks P_chunk^T seq_chunk per core; the tiny combination happens
on host in float64.

The sliding-window aggregation itself runs on the TensorEngine via a banded
constant matrix W (lhsT), chunked 121 windows at a time so each chunk's
windows only need the chunk's own 128 rows.
"""

import os

# The device run goes through jax's axon/neuron backend; a cpu-only pin
# (used for reference computations elsewhere) would hide the NeuronCores.
if os.environ.get("JAX_PLATFORMS", "").strip() == "cpu":
    del os.environ["JAX_PLATFORMS"]

import numpy as np
import ml_dtypes

B, S, m, L, RANK = 32, 2048, 128, 64, 8
Nw = S - RANK + 1  # 2041
NCORES = 8
BPC = B // NCORES  # batches per core = 4
CH = 121  # windows per chunk (window w needs rows w..w+7, so 121+7=128 rows)
NCH = (Nw + CH - 1) // CH  # 17 chunks
TAILW = Nw - (NCH - 1) * CH  # 105 windows in the last chunk
CW = BPC * m  # free columns per chunk = 512

BF16 = ml_dtypes.bfloat16

_NC_CACHE = {}

N_WARMUP_MM = 5  # dummy N=256 matmuls to warm the PE HAM clock gate


def _build_nc():
    import concourse.bacc as bacc
    import concourse.mybir as mybir
    import concourse.tile as tile

    bf = mybir.dt.bfloat16
    f32 = mybir.dt.float32
    f8 = mybir.dt.float8e4

    nc = bacc.Bacc("TRN2", target_bir_lowering=False, debug=False,
                   enable_partition_id=False)

    seq_d = nc.dram_tensor("seq", [128, NCH * CW], bf, kind="ExternalInput")
    w_d = nc.dram_tensor("wmat", [128, 2 * m], bf, kind="ExternalInput")
    p_d = nc.dram_tensor("pmat", [128, NCH * m], bf, kind="ExternalInput")
    out_d = nc.dram_tensor("out", [128, m + CW], f32, kind="ExternalOutput")

    # chunk pairs per PSUM round (17 chunks -> 8 pairs + 1 singleton)
    PAIRS = [(c, min(c + 2, NCH)) for c in range(0, NCH, 2)]
    NP = len(PAIRS)
    # seq DMA pieces (pair-aligned), in consumption order on the sync ring
    PIECES = [(0, 2), (2, 4), (4, 8), (8, 12), (12, 16), (16, NCH)]

    with tile.TileContext(nc) as tc:
        with (
            tc.tile_pool(name="const", bufs=1) as cpool,
            tc.tile_pool(name="agg", bufs=3) as apool,
            tc.tile_pool(name="psa", bufs=3, space="PSUM") as pspool,
            tc.tile_pool(name="psacc", bufs=1, space="PSUM") as accpool,
        ):
            # One HWDGE ring (sync), FIFO in consumption order:
            # wmat, seq pieces 0-1, pmat (needed from X-mm of pair 1 on),
            # then the remaining seq pieces.
            s_w = cpool.tile([128, 2 * m], bf, tag="w")
            nc.sync.dma_start(out=s_w[:], in_=w_d[:])
            s_p = cpool.tile([128, NCH * m], bf, tag="p")
            seq_tiles = [None] * NCH  # chunk -> (tile, base col)

            def load_piece(pc):
                a, b_ = PIECES[pc]
                t = cpool.tile([128, (b_ - a) * CW], bf, tag=f"seq{a}")
                nc.sync.dma_start(out=t[:], in_=seq_d[:, a * CW:b_ * CW])
                for c in range(a, b_):
                    seq_tiles[c] = (t, (c - a) * CW)

            load_piece(0)
            load_piece(1)
            nc.sync.dma_start(out=s_p[:], in_=p_d[:])
            for pc in range(2, len(PIECES)):
                load_piece(pc)

            def seq_ap(c):
                t, o = seq_tiles[c]
                return t[:, o:o + CW]

            G_ps = accpool.tile([128, m], f32, tag="G")
            X_ps = accpool.tile([128, CW], f32, tag="X")

            # PE warmup while the first seq piece is in flight; writes land
            # in X_ps and are wiped by the first real X-matmul (start=True).
            for _ in range(N_WARMUP_MM):
                nc.tensor.matmul(X_ps[:, 0:2 * m], s_w[:, 0:m], s_w[:],
                                 start=True, stop=True, skip_group_check=True)

            agg_tiles = {}

            def emit_win(p):
                c0, c1 = PAIRS[p]
                agg_ps = pspool.tile([128, 2 * CW], f32, tag="aggps")
                agg_tiles[p] = agg_ps
                for k, c in enumerate(range(c0, c1)):
                    wsel = s_w[:, 0:m] if c < NCH - 1 else s_w[:, m:2 * m]
                    nc.tensor.matmul(agg_ps[:, k * CW:(k + 1) * CW], wsel,
                                     seq_ap(c), start=True, stop=True)

            def emit_x(p):
                for c in range(*PAIRS[p]):
                    nc.tensor.matmul(
                        X_ps[:], s_p[:, c * m:(c + 1) * m], seq_ap(c),
                        start=(c == 0), stop=(c == NCH - 1),
                        skip_group_check=True,
                    )

            # software pipeline: CAST(p) overlaps PE's win(p+1)/X(p)
            emit_win(0)
            for p in range(NP):
                c0, c1 = PAIRS[p]
                n = c1 - c0
                aggb = apool.tile([128, 2 * CW], bf, tag="aggb")
                if n == 2:
                    # halves live in different PSUM banks -> DVE || ACT
                    nc.vector.tensor_copy(aggb[:, :CW], agg_tiles[p][:, :CW])
                    nc.scalar.copy(aggb[:, CW:2 * CW], agg_tiles[p][:, CW:2 * CW])
                else:
                    nc.vector.tensor_copy(aggb[:, :n * CW], agg_tiles[p][:, :n * CW])
                if p + 1 < NP:
                    emit_win(p + 1)
                emit_x(p)
                for j in range(n * BPC):
                    blk = aggb[:, j * m:(j + 1) * m]
                    nc.tensor.matmul(
                        G_ps[:], blk, blk,
                        start=(p == 0 and j == 0),
                        stop=(p == NP - 1 and j == n * BPC - 1),
                        skip_group_check=True,
                    )

            # X finishes before the last grams: copy + DMA it out early
            s_out = cpool.tile([128, m + CW], f32, tag="out")
            nc.scalar.copy(s_out[:, m:m + CW], X_ps[:])
            nc.sync.dma_start(out=out_d[:, m:m + CW], in_=s_out[:, m:m + CW])
            nc.vector.tensor_copy(s_out[:, 0:m], G_ps[:])
            nc.scalar.dma_start(out=out_d[:, 0:m], in_=s_out[:, 0:m])

    nc.compile()
    return nc


def get_nc():
    if "nc" not in _NC_CACHE:
        _NC_CACHE["nc"] = _build_nc()
    return _NC_CACHE["nc"]


def _chunk_rows():
    rows = CH * np.arange(NCH)[:, None] + np.arange(128)[None, :]  # [NCH, 128]
    valid = rows < S
    return rows, valid


def host_prep(seq_batch, M, Acoeff, Bbasis):
    """Build per-core device inputs + host-side exact terms."""
    rows, valid = _chunk_rows()
    rows_c = np.minimum(rows, S - 1)

    # seq image: per core [128, NCH, BPC, m] with seq_img[p, c, j] = seq[4k+j, 121c+p]
    g = seq_batch[:, rows_c, :].astype(BF16)  # [B, NCH, 128, m]
    g[:, ~valid, :] = 0
    imgs = np.ascontiguousarray(
        g.reshape(NCORES, BPC, NCH, 128, m).transpose(0, 3, 2, 1, 4)
    ).reshape(NCORES, 128, NCH * BPC * m)

    # banded window matrices (lhsT): out[w, n] = sum_k W[k, w] rhs[k, n]
    k = np.arange(128)[:, None]
    w = np.arange(128)[None, :]
    band = ((k - w >= 0) & (k - w < RANK)).astype(np.float32) / RANK
    wmain = band * (w < CH)
    wtail = band * (w < TAILW)
    wmat = np.concatenate([wmain, wtail], axis=1).astype(BF16)  # [128, 256]

    # Nk / Ntil / P in float64
    M64 = np.asarray(M, np.float64)
    kmod = np.arange(Nw) % L
    Nk = (np.asarray(Acoeff, np.float64).T[kmod]
          * np.asarray(Bbasis, np.float64)[kmod])  # [Nw, m]
    Ntil = Nk @ M64  # [Nw, m]
    csum = np.concatenate([np.zeros((1, m)), np.cumsum(Ntil, axis=0)])
    s = np.arange(S)
    lo = np.maximum(s - (RANK - 1), 0)
    hi = np.minimum(s, Nw - 1)
    P = (csum[hi + 1] - csum[lo]) / RANK  # [S, m]

    pr = P[rows_c].astype(np.float32)  # [NCH, 128, m]
    pvalid = valid & (np.arange(128) < CH)[None, :]
    pr[~pvalid] = 0
    pmat = np.ascontiguousarray(pr.transpose(1, 0, 2)).reshape(128, NCH * m)
    pmat = pmat.astype(BF16)

    t3 = B * float((Nk ** 2).sum())
    MtM = M64.T @ M64
    return imgs, wmat, pmat, MtM, t3


def combine(results, MtM, t3):
    """results: list of 8 arrays [128, 640] f32 -> scalar D."""
    G = np.zeros((m, m), np.float64)
    t2 = 0.0
    for r in results:
        r = np.asarray(r, np.float64)
        G += r[:, :m]
        for j in range(BPC):
            t2 += np.trace(r[:, m + j * m:m + (j + 1) * m])
    t1 = float((MtM * G).sum())
    D = (t1 - 2.0 * t2 + t3) / (B * Nw * m)
    return np.float32(D)


def kernel(seq_batch, M, Acoeff, Bbasis):
    from concourse.bass_utils import run_bass_kernel_spmd

    seq_batch = np.asarray(seq_batch, np.float32)
    imgs, wmat, pmat, MtM, t3 = host_prep(seq_batch, M, Acoeff, Bbasis)

    nc = get_nc()
    in_maps = [
        {"seq": imgs[c], "wmat": wmat, "pmat": pmat} for c in range(NCORES)
    ]
    res = run_bass_kernel_spmd(nc, in_maps, core_ids=list(range(NCORES)))
    outs = [res.results[c]["out"] for c in range(NCORES)]
    return combine(outs, MtM, t3)



# revision 2
# speedup vs baseline: 1.0916x; 1.0916x over previous
"""Trainium2 Bass kernel for nn_NumDualDescriptorAB.

Reference computation:
    agg[b,w]   = mean(seq[b, w:w+8, :], axis=0)          (sliding window, Nw = S-7)
    y[b,w]     = agg[b,w] @ M.T
    Nk[w]      = Acoeff[:, w%L] * Bbasis[w%L, :]
    D          = mean((y - Nk)^2)

Algebraic decomposition:
    count = B*Nw*m
    t1 = sum_{b,w} agg MtM agg^T = <M^T M, G>_F   with G = sum agg^T agg   (m x m)
    t2 = sum_{b,w} y . Nk = sum_s seqsum[s] . P[s]   with seqsum = sum_b seq[b],
         P = W^T (Nk M)  -- LINEAR in seq, so it is a trivial host reduction.
    t3 = B * ||Nk||^2
    D  = (t1 - 2 t2 + t3) / count

Only the quadratic term G needs the device.  Each of the 8 cores handles 4
batches: sliding windows come from a banded constant matrix W (lhsT) applied
to 128-row seq chunks (121 windows per chunk, 17 chunks), the agg chunk is
cast PSUM->SBUF bf16 (DVE/ACT alternating), and per-batch Gram matmuls
accumulate G in a PSUM bank.  Dummy matmuls at kernel start keep the PE HAM
activity window busy during the DMA fill so the clock un-throttles (K=8/8)
just as the real work ramps.

Host side (float64): P/seqsum/t2, t3, M^T M, and the final combine.
"""

import os

# The device run goes through jax's axon/neuron backend; a cpu-only pin
# (used for reference computations elsewhere) would hide the NeuronCores.
if os.environ.get("JAX_PLATFORMS", "").strip() == "cpu":
    del os.environ["JAX_PLATFORMS"]

import numpy as np
import ml_dtypes

B, S, m, L, RANK = 32, 2048, 128, 64, 8
Nw = S - RANK + 1  # 2041
NCORES = 8
BPC = B // NCORES  # batches per core = 4
CH = 121  # windows per chunk (window w needs rows w..w+7, so 121+7=128 rows)
NCH = (Nw + CH - 1) // CH  # 17 chunks
TAILW = Nw - (NCH - 1) * CH  # 105 windows in the last chunk
CW = BPC * m  # free columns per chunk = 512

BF16 = ml_dtypes.bfloat16

_NC_CACHE = {}

N_DUMMY = 12  # N=128 dummy matmuls to warm the PE HAM clock gate during DMA
# seq DMA pieces (chunk counts), all on the sync HWDGE ring in consumption
# order; small first piece so the first window matmul starts early.
PIECES = [1, 2, 3, 3, 4, 4]


def _build_nc():
    import concourse.bacc as bacc
    import concourse.mybir as mybir
    import concourse.tile as tile

    bf = mybir.dt.bfloat16
    f32 = mybir.dt.float32

    nc = bacc.Bacc("TRN2", target_bir_lowering=False, debug=False,
                   enable_partition_id=False)

    seq_d = nc.dram_tensor("seq", [128, NCH * CW], bf, kind="ExternalInput")
    w_d = nc.dram_tensor("wmat", [128, 2 * m], bf, kind="ExternalInput")
    out_d = nc.dram_tensor("out", [128, m], f32, kind="ExternalOutput")

    with tile.TileContext(nc) as tc:
        with (
            tc.tile_pool(name="const", bufs=1) as cpool,
            tc.tile_pool(name="aggb", bufs=3) as apool,
            tc.tile_pool(name="psa", bufs=4, space="PSUM") as pspool,
            tc.tile_pool(name="psacc", bufs=1, space="PSUM") as accpool,
        ):
            # --- DMA issue (t=0): wmat on the scalar ring, seq pieces on the
            # sync ring (serial issue, arrival order == consumption order).
            s_w = cpool.tile([128, 2 * m], bf, tag="w")
            nc.scalar.dma_start(out=s_w[:], in_=w_d[:])

            seq_tiles = [None] * NCH  # chunk -> (tile, base col)
            a = 0
            for pc, n in enumerate(PIECES):
                t = cpool.tile([128, n * CW], bf, tag=f"seq{pc}",
                               name=f"seqp{pc}")
                nc.sync.dma_start(out=t[:], in_=seq_d[:, a * CW:(a + n) * CW])
                for c in range(a, a + n):
                    seq_tiles[c] = (t, (c - a) * CW)
                a += n

            def seq_ap(c):
                t, o = seq_tiles[c]
                return t[:, o:o + CW]

            G_ps = accpool.tile([128, m], f32, tag="G")
            scr_ps = accpool.tile([128, m], f32, tag="scr")

            # --- PE warmup: no data dependencies, keeps the HAM activity
            # window busy while the first seq piece is in flight.
            dum = cpool.tile([128, m], bf, tag="dum")
            nc.gpsimd.memset(dum[:], 0)
            for _ in range(N_DUMMY):
                nc.tensor.matmul(scr_ps[:], dum[:], dum[:],
                                 start=True, stop=True, skip_group_check=True)

            # --- main pipeline: win(c) -> cast(c) -> 4 gram matmuls
            agg_tiles = {}

            def emit_win(c):
                agg_ps = pspool.tile([128, CW], f32, tag="aggps",
                                     name=f"agg{c}")
                agg_tiles[c] = agg_ps
                wsel = s_w[:, 0:m] if c < NCH - 1 else s_w[:, m:2 * m]
                nc.tensor.matmul(agg_ps[:], wsel, seq_ap(c),
                                 start=True, stop=True)

            emit_win(0)
            emit_win(1)
            for c in range(NCH):
                aggb = apool.tile([128, CW], bf, tag="aggb", name=f"aggb{c}")
                eng = nc.vector if c % 2 == 0 else nc.scalar
                if c % 2 == 0:
                    eng.tensor_copy(aggb[:], agg_tiles[c][:])
                else:
                    eng.copy(aggb[:], agg_tiles[c][:])
                if c + 2 < NCH:
                    emit_win(c + 2)
                for j in range(BPC):
                    blk = aggb[:, j * m:(j + 1) * m]
                    nc.tensor.matmul(
                        G_ps[:], blk, blk,
                        start=(c == 0 and j == 0),
                        stop=(c == NCH - 1 and j == BPC - 1),
                        skip_group_check=True,
                    )

            s_out = cpool.tile([128, m], f32, tag="out")
            nc.vector.tensor_copy(s_out[:], G_ps[:])
            nc.sync.dma_start(out=out_d[:], in_=s_out[:])

    nc.compile()
    return nc


def get_nc():
    if "nc" not in _NC_CACHE:
        _NC_CACHE["nc"] = _build_nc()
    return _NC_CACHE["nc"]


def _chunk_rows():
    rows = CH * np.arange(NCH)[:, None] + np.arange(128)[None, :]  # [NCH, 128]
    valid = rows < S
    return rows, valid


def host_prep(seq_batch, M, Acoeff, Bbasis):
    """Build per-core device inputs + host-side exact terms."""
    rows, valid = _chunk_rows()
    rows_c = np.minimum(rows, S - 1)

    # seq image: per core [128, NCH, BPC, m] with seq_img[p, c, j] = seq[4k+j, 121c+p]
    g = seq_batch[:, rows_c, :].astype(BF16)  # [B, NCH, 128, m]
    g[:, ~valid, :] = 0
    imgs = np.ascontiguousarray(
        g.reshape(NCORES, BPC, NCH, 128, m).transpose(0, 3, 2, 1, 4)
    ).reshape(NCORES, 128, NCH * BPC * m)

    # banded window matrices (lhsT): out[w, n] = sum_k W[k, w] rhs[k, n]
    k = np.arange(128)[:, None]
    w = np.arange(128)[None, :]
    band = ((k - w >= 0) & (k - w < RANK)).astype(np.float32) / RANK
    wmain = band * (w < CH)
    wtail = band * (w < TAILW)
    wmat = np.concatenate([wmain, wtail], axis=1).astype(BF16)  # [128, 256]

    # linear terms in float64 on host: t2 = <seqsum, P>, t3 = B*||Nk||^2
    M64 = np.asarray(M, np.float64)
    kmod = np.arange(Nw) % L
    Nk = (np.asarray(Acoeff, np.float64).T[kmod]
          * np.asarray(Bbasis, np.float64)[kmod])  # [Nw, m]
    Ntil = Nk @ M64  # [Nw, m]
    csum = np.concatenate([np.zeros((1, m)), np.cumsum(Ntil, axis=0)])
    s = np.arange(S)
    lo = np.maximum(s - (RANK - 1), 0)
    hi = np.minimum(s, Nw - 1)
    P = (csum[hi + 1] - csum[lo]) / RANK  # [S, m]

    seqsum = np.asarray(seq_batch, np.float64).sum(axis=0)  # [S, m]
    t2 = float((seqsum * P).sum())
    t3 = B * float((Nk ** 2).sum())
    MtM = M64.T @ M64
    return imgs, wmat, MtM, t2, t3


def combine(results, MtM, t2, t3):
    """results: list of 8 arrays [128, 128] f32 (per-core G) -> scalar D."""
    G = np.zeros((m, m), np.float64)
    for r in results:
        G += np.asarray(r, np.float64)
    t1 = float((MtM * G).sum())
    D = (t1 - 2.0 * t2 + t3) / (B * Nw * m)
    return np.float32(D)


def kernel(seq_batch, M, Acoeff, Bbasis):
    from concourse.bass_utils import run_bass_kernel_spmd

    seq_batch = np.asarray(seq_batch, np.float32)
    imgs, wmat, MtM, t2, t3 = host_prep(seq_batch, M, Acoeff, Bbasis)

    nc = get_nc()
    in_maps = [
        {"seq": imgs[c], "wmat": wmat} for c in range(NCORES)
    ]
    res = run_bass_kernel_spmd(nc, in_maps, core_ids=list(range(NCORES)))
    outs = [res.results[c]["out"] for c in range(NCORES)]
    return combine(outs, MtM, t2, t3)


# revision 6
# speedup vs baseline: 1.1379x; 1.0424x over previous
"""Trainium2 Bass kernel for nn_NumDualDescriptorAB.

Reference computation:
    agg[b,w]   = mean(seq[b, w:w+8, :], axis=0)          (sliding window, Nw = S-7)
    y[b,w]     = agg[b,w] @ M.T
    Nk[w]      = Acoeff[:, w%L] * Bbasis[w%L, :]
    D          = mean((y - Nk)^2)

Algebraic decomposition:
    count = B*Nw*m
    t1 = sum_{b,w} agg MtM agg^T = <M^T M, G>_F   with G = sum agg^T agg   (m x m)
    t2 = sum_{b,w} y . Nk = sum_s seqsum[s] . P[s]   with seqsum = sum_b seq[b],
         P = W^T (Nk M)  -- LINEAR in seq, so it is a trivial host reduction.
    t3 = B * ||Nk||^2
    D  = (t1 - 2 t2 + t3) / count

Only the quadratic term G needs the device.  Each of the 8 cores handles 4
batches: sliding windows come from a banded constant matrix W (lhsT) applied
to 128-row seq chunks (121 windows per chunk, 17 chunks), the agg chunk is
cast PSUM->SBUF bf16 (split between DVE and ACT), and per-batch Gram matmuls
accumulate G in a PSUM bank.

Schedule notes (from NTFF traces):
  - The PE HAM clock gate needs ~3.4us of sustained activity before the PE
    un-throttles from 1.2 to 2.4 GHz; 1-column dummy matmuls with no data
    dependencies keep the PE busy from t~1us while the first DMA piece is
    in flight (HBM completion receipt alone is ~2us).
  - Steady state is LDWEIGHTS-rate bound (~5 loads x ~128ns per chunk), so
    PSUM depth 6 + deep window prefill just needs to keep bubbles out.
  - The final G DMA is issued OUTSIDE the TileContext: nothing waits on its
    completion semaphore, so the ~2us HBM write receipt overlaps the fixed
    walrus teardown (per-engine semaphore-clear chains) instead of
    preceding it.  The teardown takes ~7us, far longer than the receipt.

Host side (float64): P/seqsum/t2, t3, M^T M, and the final combine.
"""

import os

# The device run goes through jax's axon/neuron backend; a cpu-only pin
# (used for reference computations elsewhere) would hide the NeuronCores.
if os.environ.get("JAX_PLATFORMS", "").strip() == "cpu":
    del os.environ["JAX_PLATFORMS"]

import numpy as np
import ml_dtypes

B, S, m, L, RANK = 32, 2048, 128, 64, 8
Nw = S - RANK + 1  # 2041
NCORES = 8
BPC = B // NCORES  # batches per core = 4
CH = 121  # windows per chunk (window w needs rows w..w+7, so 121+7=128 rows)
NCH = (Nw + CH - 1) // CH  # 17 chunks
TAILW = Nw - (NCH - 1) * CH  # 105 windows in the last chunk
CW = BPC * m  # free columns per chunk = 512
WCOLS = 2 * m  # wmat columns (wmain | wtail), stored ahead of seq data

BF16 = ml_dtypes.bfloat16

_NC_CACHE = {}

N_DUMMY = 22  # 1-column dummy matmuls to warm the PE HAM clock gate
# seq DMA pieces as column ranges of the combined [wmat | chunks] tensor,
# all on the sync HWDGE ring in consumption order; the first piece (wmat +
# chunk 0) is small so the first window matmul starts early.
PIECE_CHUNKS = [1, 2, 3, 3, 4, 4]


def _build_nc():
    import concourse.bacc as bacc
    import concourse.mybir as mybir
    import concourse.tile as tile

    bf = mybir.dt.bfloat16
    f32 = mybir.dt.float32

    nc = bacc.Bacc("TRN2", target_bir_lowering=False, debug=False,
                   enable_partition_id=False)

    seq_d = nc.dram_tensor("seq", [128, WCOLS + NCH * CW], bf,
                           kind="ExternalInput")
    out_d = nc.dram_tensor("out", [128, m], f32, kind="ExternalOutput")

    # raw (non-tile) SBUF tensor so the fire-and-forget DMA below has a
    # concrete access pattern
    s_out = nc.alloc_sbuf_tensor("s_out", [128, m], f32)

    with tile.TileContext(nc) as tc:
        with (
            tc.tile_pool(name="const", bufs=1) as cpool,
            tc.tile_pool(name="aggb", bufs=3) as apool,
            tc.tile_pool(name="psa", bufs=6, space="PSUM") as pspool,
            tc.tile_pool(name="psacc", bufs=1, space="PSUM") as accpool,
        ):
            # --- DMA issue (t=0): all pieces on the sync ring, serial issue
            # so arrival order == consumption order.  Piece 0 carries wmat.
            s_w = cpool.tile([128, WCOLS], bf, tag="w")
            seq_tiles = [None] * NCH  # chunk -> (tile, base col)
            a = 0
            for pc, n in enumerate(PIECE_CHUNKS):
                t = cpool.tile([128, n * CW], bf, tag=f"seq{pc}",
                               name=f"seqp{pc}")
                if pc == 0:
                    # one DMA covers wmat + the first chunk
                    nc.sync.dma_start(out=s_w[:], in_=seq_d[:, 0:WCOLS])
                nc.sync.dma_start(
                    out=t[:],
                    in_=seq_d[:, WCOLS + a * CW:WCOLS + (a + n) * CW])
                for c in range(a, a + n):
                    seq_tiles[c] = (t, (c - a) * CW)
                a += n

            def seq_ap(c):
                t, o = seq_tiles[c]
                return t[:, o:o + CW]

            G_ps = accpool.tile([128, m], f32, tag="G")
            scr_ps = accpool.tile([128, m], f32, tag="scr")

            # --- PE warmup: no data dependencies, 1-column weights so the
            # LDW path stays free; keeps the HAM activity window busy while
            # the first seq piece is in flight.
            dum = cpool.tile([128, m], bf, tag="dum")
            nc.gpsimd.memset(dum[:], 0)
            for _ in range(N_DUMMY):
                nc.tensor.matmul(scr_ps[0:1, :], dum[:, 0:1], dum[:],
                                 start=True, stop=True, skip_group_check=True)

            # --- main pipeline: win(c) -> cast(c) (DVE half + ACT half)
            # -> 4 gram matmuls
            agg_tiles = {}

            def emit_win(c):
                agg_ps = pspool.tile([128, CW], f32, tag="aggps",
                                     name=f"agg{c}")
                agg_tiles[c] = agg_ps
                wsel = s_w[:, 0:m] if c < NCH - 1 else s_w[:, m:2 * m]
                nc.tensor.matmul(agg_ps[:], wsel, seq_ap(c),
                                 start=True, stop=True)

            NPRE = 6
            for c in range(NPRE):
                emit_win(c)
            for c in range(NCH):
                aggb = apool.tile([128, CW], bf, tag="aggb", name=f"aggb{c}")
                half = CW // 2
                nc.vector.tensor_copy(aggb[:, 0:half],
                                      agg_tiles[c][:, 0:half])
                nc.scalar.copy(aggb[:, half:CW], agg_tiles[c][:, half:CW])
                if c + NPRE < NCH:
                    emit_win(c + NPRE)
                for j in range(BPC):
                    blk = aggb[:, j * m:(j + 1) * m]
                    nc.tensor.matmul(
                        G_ps[:], blk, blk,
                        start=(c == 0 and j == 0),
                        stop=(c == NCH - 1 and j == BPC - 1),
                        skip_group_check=True,
                    )

            nc.scalar.copy(s_out.ap(), G_ps[:])
            nc.sync.dma_start(out=out_d[:], in_=s_out.ap())

    nc.compile()
    return nc


def get_nc():
    if "nc" not in _NC_CACHE:
        _NC_CACHE["nc"] = _build_nc()
    return _NC_CACHE["nc"]


def _chunk_rows():
    rows = CH * np.arange(NCH)[:, None] + np.arange(128)[None, :]  # [NCH, 128]
    valid = rows < S
    return rows, valid


def host_prep(seq_batch, M, Acoeff, Bbasis):
    """Build per-core device inputs + host-side exact terms."""
    rows, valid = _chunk_rows()
    rows_c = np.minimum(rows, S - 1)

    # seq image: per core [128, NCH, BPC, m] with seq_img[p, c, j] = seq[4k+j, 121c+p]
    g = seq_batch[:, rows_c, :].astype(BF16)  # [B, NCH, 128, m]
    g[:, ~valid, :] = 0
    imgs = np.ascontiguousarray(
        g.reshape(NCORES, BPC, NCH, 128, m).transpose(0, 3, 2, 1, 4)
    ).reshape(NCORES, 128, NCH * BPC * m)

    # banded window matrices (lhsT): out[w, n] = sum_k W[k, w] rhs[k, n]
    k = np.arange(128)[:, None]
    w = np.arange(128)[None, :]
    band = ((k - w >= 0) & (k - w < RANK)).astype(np.float32) / RANK
    wmain = band * (w < CH)
    wtail = band * (w < TAILW)
    wmat = np.concatenate([wmain, wtail], axis=1).astype(BF16)  # [128, 256]

    # combined device input: [wmat | seq chunks]
    full = np.concatenate(
        [np.broadcast_to(wmat, (NCORES, 128, WCOLS)), imgs], axis=2)
    full = np.ascontiguousarray(full)

    # linear terms in float64 on host: t2 = <seqsum, P>, t3 = B*||Nk||^2
    M64 = np.asarray(M, np.float64)
    kmod = np.arange(Nw) % L
    Nk = (np.asarray(Acoeff, np.float64).T[kmod]
          * np.asarray(Bbasis, np.float64)[kmod])  # [Nw, m]
    Ntil = Nk @ M64  # [Nw, m]
    csum = np.concatenate([np.zeros((1, m)), np.cumsum(Ntil, axis=0)])
    s = np.arange(S)
    lo = np.maximum(s - (RANK - 1), 0)
    hi = np.minimum(s, Nw - 1)
    P = (csum[hi + 1] - csum[lo]) / RANK  # [S, m]

    seqsum = np.asarray(seq_batch, np.float64).sum(axis=0)  # [S, m]
    t2 = float((seqsum * P).sum())
    t3 = B * float((Nk ** 2).sum())
    MtM = M64.T @ M64
    return full, MtM, t2, t3


def combine(results, MtM, t2, t3):
    """results: list of 8 arrays [128, 128] f32 (per-core G) -> scalar D."""
    G = np.zeros((m, m), np.float64)
    for r in results:
        G += np.asarray(r, np.float64)
    t1 = float((MtM * G).sum())
    D = (t1 - 2.0 * t2 + t3) / (B * Nw * m)
    return np.float32(D)


def kernel(seq_batch, M, Acoeff, Bbasis):
    from concourse.bass_utils import run_bass_kernel_spmd

    seq_batch = np.asarray(seq_batch, np.float32)
    full, MtM, t2, t3 = host_prep(seq_batch, M, Acoeff, Bbasis)

    nc = get_nc()
    in_maps = [{"seq": full[c]} for c in range(NCORES)]
    res = run_bass_kernel_spmd(nc, in_maps, core_ids=list(range(NCORES)))
    outs = [res.results[c]["out"] for c in range(NCORES)]
    return combine(outs, MtM, t2, t3)
